# revision 27
# baseline (speedup 1.0000x reference)
"""2-layer GraphSAGE (mean aggregation) on 8 trn2 NeuronCores via Bass/Tile.

Strategy (matches the sharding hint):
  - Nodes are row-sharded across the 8 cores (6250 rows each); edges are
    partitioned by destination core and grouped by 128-node destination
    block.  Messages are fetched with InstDMAGatherAnt from a pair-packed
    bf16 copy of the features: each 256B gather element holds TWO
    consecutive node rows (128B each) and the index is src>>1, so no
    parity split of the edges is needed.  Gather calls round-robin over
    4 SWDGE queues (4 DMA engines).
  - The segment-sum is a one-hot matmul on the tensor engine (bf16).
    Each gathered tile is reduced with TWO matmuls — the even-node
    column half against an even-masked onehot and the odd half against
    an odd-masked onehot — accumulating into one PSUM tile:
        aggT[64f, 128d] += msgs[128e, 64(v)].T @ onehot_v[128e, 128d]
    A single broadcast-AP DVE tensor_tensor(is_equal) against an
    interleaved host-built dstf table builds BOTH onehot variants for a
    whole gather call in one instruction.  Exact f32 1/deg scaling is
    applied at PSUM->SBUF copy time via a host-built broadcast table.
  - The 64x64 weights are replicated; the dense phase runs feature-major
    in f32 on rotating [64, 512] group buffers.
  - h = tanh(layer1) is written pair-packed (128B rows, no padding) and
    AllGathered in node-range chunks that overlap the remaining layer-1
    compute; layer 2 gathers from the chunk-major shared h with the SAME
    edge grouping (pair index pos>>1, parity preserved because all
    layout offsets are even).
"""

import numpy as np
import ml_dtypes

import concourse.bacc as bacc
import concourse.mybir as mybir
import concourse.tile as tile
from concourse.bass import AP
from concourse.bass_utils import run_bass_kernel_spmd

P = 128
D = 64
F32 = mybir.dt.float32
BF16 = mybir.dt.bfloat16
I16 = mybir.dt.int16
BF = ml_dtypes.bfloat16

GCOL = 512  # dense-phase group width (one PSUM bank)


class Cfg:
    def __init__(self, N, n_cores=8, chunk=24, msgs_bufs=6, nqueues=4,
                 oh_bufs=3, n_hchunks=4):
        assert N % n_cores == 0
        self.N = N
        self.n_cores = n_cores
        self.n_own = N // n_cores
        assert self.n_own % 2 == 0
        self.nblk = -(-self.n_own // P)
        self.n_own_pad = self.nblk * P
        self.n_pad_all = self.n_own_pad * n_cores
        self.chunk = chunk
        self.msgs_bufs = msgs_bufs
        self.nqueues = nqueues
        self.oh_bufs = oh_bufs
        # h chunks: contiguous ranges of dense groups (GCOL-node units)
        ngrp = -(-self.nblk * P // GCOL)
        gper = -(-ngrp // n_hchunks)
        self.hchunks = []
        g0 = 0
        while g0 < ngrp:
            g1 = min(g0 + gper, ngrp)
            base = g0 * GCOL
            sz = min(g1 * GCOL, self.n_own_pad) - base
            self.hchunks.append((base, sz, g0, g1))
            g0 = g1
        assert all(b % 2 == 0 and s % 2 == 0 for b, s, _, _ in self.hchunks)
        assert self.n_own_pad // 2 * n_cores <= 32768  # int16 pair idx


class Meta:
    pass


def _wrap16(v):
    """slot i -> [i % 16, i // 16] layout used by dma_gather idx tables."""
    assert v.shape[0] % 16 == 0
    return np.ascontiguousarray(v.reshape(-1, 16).T)


def preprocess(edge_index, cfg):
    """Partition/group edges; build per-core gather index + onehot tables."""
    src = np.asarray(edge_index[0], dtype=np.int64)
    dst = np.asarray(edge_index[1], dtype=np.int64)
    E = src.shape[0]
    NC, NBLK = cfg.n_cores, cfg.nblk

    cnt = np.bincount(dst, minlength=cfg.N).astype(np.float32)
    inv = (1.0 / np.maximum(cnt, 1.0)).astype(np.float32)

    core = dst // cfg.n_own
    dstl = dst - core * cfg.n_own
    blk = dstl // P
    inb = dstl - blk * P
    par = (src & 1).astype(np.int64)

    key = core * NBLK + blk
    gcnt = np.bincount(key, minlength=NC * NBLK).reshape(NC, NBLK)
    # uniform (max over cores) tile counts per block
    TB = np.maximum(1, -(-gcnt.max(axis=0) // P))
    toff = np.concatenate([[0], np.cumsum(TB)])
    T_ALL = int(toff[-1])

    # rank of each edge within its (core, blk) group
    order = np.argsort(key, kind="stable")
    gstart = np.concatenate(
        [[0], np.cumsum(np.bincount(key, minlength=NC * NBLK))])[:-1]
    rank = np.empty(E, dtype=np.int64)
    rank[order] = np.arange(E) - gstart[key[order]]
    slot = toff[blk] * P + rank
    tile_of = slot >> 7

    # h-space (chunk-major, padded) position of each source node
    src_core = src // cfg.n_own
    local = src - src_core * cfg.n_own
    bases = np.array([b for b, _, _, _ in cfg.hchunks], dtype=np.int64)
    sizes = np.array([s for _, s, _, _ in cfg.hchunks], dtype=np.int64)
    ci = np.searchsorted(bases, local, side="right") - 1
    pos = NC * bases[ci] + src_core * sizes[ci] + (local - bases[ci])

    # SPMD-uniform per-(tile, parity-variant) activity across all cores
    act = np.zeros((T_ALL, 2), dtype=bool)
    act[tile_of, par] = True

    meta = Meta()
    meta.cfg = cfg
    meta.T_ALL = T_ALL
    meta.toff = toff
    meta.block_tiles = []
    for b in range(NBLK):
        bt = [(int(t), v) for t in range(int(toff[b]), int(toff[b + 1]))
              for v in (0, 1) if act[t, v]]
        if not bt:
            bt = [(int(toff[b]), 0)]
        meta.block_tiles.append(bt)

    # per-core tables
    meta.idx = []    # [128, 8*T_ALL*2] int16 : layer1 | layer2 (pair indices)
    meta.dstf = []   # [128, 2*T_ALL] bf16, col 2t+v = onehot targets, -1 pad
    meta.invb = []   # [64, n_own_pad] f32 : 1/deg broadcast down 64 partitions
    for k in range(NC):
        m = core == k
        sl = slot[m]
        i1 = np.zeros(T_ALL * P, np.int16)
        i2 = np.zeros(T_ALL * P, np.int16)
        i1[sl] = src[m] >> 1
        i2[sl] = pos[m] >> 1
        w = np.concatenate([_wrap16(i1), _wrap16(i2)], axis=1)
        # the gather ucode reads each Q7 core's idx stripe from its own
        # 16-partition group -> replicate 8x down the partition axis
        meta.idx.append(np.ascontiguousarray(np.tile(w, (8, 1))))

        df = np.full(2 * T_ALL * P, -1.0, BF)
        gi = (sl >> 7) * 2 + par[m]
        df[gi * P + (sl & (P - 1))] = inb[m].astype(BF)
        meta.dstf.append(np.ascontiguousarray(df.reshape(2 * T_ALL, P).T))

        iv = np.ones(cfg.n_own_pad, np.float32)
        iv[:cfg.n_own] = inv[k * cfg.n_own:(k + 1) * cfg.n_own]
        meta.invb.append(np.ascontiguousarray(np.tile(iv, (D, 1))))

    # dma_gather calls: chunks of tiles, annotated with the first block
    calls = []
    t0 = 0
    while t0 < T_ALL:
        nt = min(cfg.chunk, T_ALL - t0)
        fb = int(np.searchsorted(toff, t0, side="right") - 1)
        calls.append((t0, nt, fb))
        t0 += nt
    meta.calls = calls
    return meta


def build_program(meta, one_core=False,
                  parts=("gather", "agg", "dense", "store", "collective"),
                  reps=1):
    cfg = meta.cfg
    NC, NBLK = cfg.n_cores, cfg.nblk
    NP = cfg.n_own_pad
    BPG = GCOL // P  # blocks per dense group
    nc = bacc.Bacc(
        "TRN2", target_bir_lowering=False, debug=False,
        num_devices=1 if one_core else NC,
        num_swdge_queues=cfg.nqueues,
    )

    xp_dr = nc.dram_tensor("xp", [cfg.N // 2, P], BF16, kind="ExternalInput")
    xoT_dr = nc.dram_tensor("xoT", [D, NP], F32, kind="ExternalInput")
    idx_dr = nc.dram_tensor("idx", list(meta.idx[0].shape), I16, kind="ExternalInput")
    dstf_dr = nc.dram_tensor("dstf", [P, 2 * meta.T_ALL], BF16, kind="ExternalInput")
    invb_dr = nc.dram_tensor("invb", [D, NP], F32, kind="ExternalInput")
    wl1_dr = nc.dram_tensor("wl1t", [D, D], F32, kind="ExternalInput")
    wr1_dr = nc.dram_tensor("wr1t", [D, D], F32, kind="ExternalInput")
    wl2_dr = nc.dram_tensor("wl2t", [D, D], F32, kind="ExternalInput")
    wr2_dr = nc.dram_tensor("wr2t", [D, D], F32, kind="ExternalInput")
    b1_dr = nc.dram_tensor("b1", [D, 1], F32, kind="ExternalInput")
    b2_dr = nc.dram_tensor("b2", [D, 1], F32, kind="ExternalInput")
    iota_dr = nc.dram_tensor("iota", [P, 2 * P], BF16, kind="ExternalInput")
    id_dr = nc.dram_tensor("ident", [D, D], F32, kind="ExternalInput")
    out_dr = nc.dram_tensor("out", [NP, D], F32, kind="ExternalOutput")

    with tile.TileContext(nc) as tc:
        with (
            tc.tile_pool(name="const", bufs=1) as cpool,
            tc.tile_pool(name="big", bufs=1) as bpool,
            tc.tile_pool(name="msgs", bufs=cfg.msgs_bufs) as mpool,
            tc.tile_pool(name="idxp", bufs=4) as ipool,
            tc.tile_pool(name="ohp", bufs=cfg.oh_bufs) as ohpool,
            tc.tile_pool(name="grp", bufs=2) as gpool,
            tc.tile_pool(name="psA", bufs=4, space="PSUM") as psA,
            tc.tile_pool(name="psZ", bufs=2, space="PSUM") as psZ,
            tc.tile_pool(name="psT", bufs=2, space="PSUM") as psT,
            tc.tile_pool(name="dram", bufs=1, space="DRAM") as dpool,
        ):
            def load(pool, dr, shape, name, dt=F32, tag=""):
                t = pool.tile(shape, dt, name=name, tag=tag or name)
                nc.sync.dma_start(out=t, in_=dr.ap())
                return t

            iota_sb = load(cpool, iota_dr, [P, 2 * P], "iota_sb", dt=BF16)
            ident_sb = load(cpool, id_dr, [D, D], "ident_sb")
            wl1_sb = load(cpool, wl1_dr, [D, D], "wl1_sb")
            wr1_sb = load(cpool, wr1_dr, [D, D], "wr1_sb")
            wl2_sb = load(cpool, wl2_dr, [D, D], "wl2_sb")
            wr2_sb = load(cpool, wr2_dr, [D, D], "wr2_sb")
            b1_sb = load(cpool, b1_dr, [D, 1], "b1_sb")
            b2_sb = load(cpool, b2_dr, [D, 1], "b2_sb")
            dstf_sb = load(bpool, dstf_dr, [P, 2 * meta.T_ALL], "dstf_sb", dt=BF16)
            invb_sb = load(bpool, invb_dr, [D, NP], "invb_sb")
            xoT_sb = load(bpool, xoT_dr, [D, NP], "xoT_sb")
            hT_sb = bpool.tile([D, NP], F32, name="hT_sb")
            nodeh_sb = bpool.tile([P, NBLK * D], BF16, name="nodeh_sb")
            nodeo_sb = bpool.tile([P, NBLK * D], F32, name="nodeo_sb")

            for rep in range(reps):
              h_full = dpool.tile([cfg.n_pad_all // 2, P], BF16,
                                  name=f"h_full_{rep}", tag=f"hf{rep}")
              hcks = []
              haggs = []
              for c, (base, sz, _, _) in enumerate(cfg.hchunks):
                  hcks.append(dpool.tile([sz // 2, P], BF16,
                                         name=f"h_c_{rep}_{c}", tag=f"hc{rep}_{c}"))
                  haggs.append(dpool.tile([NC * sz // 2, P], BF16,
                                          name=f"h_a_{rep}_{c}",
                                          tag=f"ha{rep}_{c}",
                                          addr_space="Shared"))
              for layer in range(2):
                if layer == 0:
                    gsrc = xp_dr.ap()
                    off = 0
                else:
                    gsrc = h_full[:, :]
                    off = meta.T_ALL * 8

                # ---- gather messages (256B elems = node pairs) ----
                tsrc = {}
                ohsrc = {}
                for ci, (t0, ntile, _fb) in enumerate(meta.calls):
                    mt = mpool.tile([P, cfg.chunk, P], BF16, tag="msgs",
                                    name=f"m_{layer}_{ci}")
                    if "gather" in parts:
                        it = ipool.tile([P, cfg.chunk * 8], I16, tag="idx",
                                        name=f"i_{layer}_{ci}")
                        cols = ntile * 8
                        coff = off + t0 * 8
                        nc.sync.dma_start(out=it[:, :cols],
                                          in_=idx_dr.ap()[:, coff:coff + cols])
                        nc.gpsimd.dma_gather(
                            mt[:, :ntile, :],
                            gsrc,
                            it[:, :cols],
                            num_idxs=ntile * P,
                            num_idxs_reg=ntile * P,
                            elem_size=P,
                            single_packet=False,
                            queue_num=ci % cfg.nqueues,
                        )
                    # one broadcast-AP DVE op builds BOTH onehot variants for
                    # this whole chunk, variant-interleaved so every operand's
                    # innermost dim is packed (keeps the DVE 2x 16-bit mode):
                    #   oh[e, t, d, v] = (iota2[e, 2d+v] == dstf[e, 2(t0+t)+v])
                    oht = ohpool.tile([P, cfg.chunk, P, 2], BF16, tag="oh",
                                      name=f"oh_{layer}_{ci}")
                    if "agg" in parts:
                        io = iota_sb[:, :]
                        in0 = AP(io.tensor, io.offset,
                                 [io.ap[0], [0, ntile], [2, P], [1, 2]])
                        df = dstf_sb[:, 2 * t0:2 * (t0 + ntile)]
                        in1 = AP(df.tensor, df.offset,
                                 [df.ap[0], [2, ntile], [0, P], [1, 2]])
                        nc.vector.tensor_tensor(
                            out=oht[:, :ntile, :, :], in0=in0, in1=in1,
                            op=mybir.AluOpType.is_equal,
                        )
                    for j in range(ntile):
                        tsrc[t0 + j] = (mt, j)
                        ohsrc[t0 + j] = (oht, j)

                # ---- blocks: onehot matmul segment-sum + dense per group ----
                if layer == 0:
                    wl_sb, wr_sb, bb_sb = wl1_sb, wr1_sb, b1_sb
                    own_sb = xoT_sb
                    func = mybir.ActivationFunctionType.Tanh
                else:
                    wl_sb, wr_sb, bb_sb = wl2_sb, wr2_sb, b2_sb
                    own_sb = hT_sb
                    func = mybir.ActivationFunctionType.Identity

                ngrp = -(-NBLK // BPG)
                grp_done = {}
                for g in range(ngrp if "agg" in parts else 0):
                    b0 = g * BPG
                    nb = min(BPG, NBLK - b0)
                    w = nb * P
                    aggT = gpool.tile([D, GCOL], F32, tag="aggT",
                                      name=f"agg_{rep}_{layer}_{g}")
                    for bi in range(nb):
                        b = b0 + bi
                        ps = psA.tile([D, P], F32, tag="agg", name=f"ps_{layer}_{b}")
                        gts = meta.block_tiles[b]
                        for j, (gt, v) in enumerate(gts):
                            mt, lt = tsrc[gt]
                            oht, lo = ohsrc[gt]
                            nc.tensor.matmul(
                                ps, lhsT=mt[:, lt, v * D:v * D + D],
                                rhs=oht[:, lo, :, v],
                                start=(j == 0), stop=(j == len(gts) - 1),
                            )
                        # exact mean scaling: psum * (1/deg) broadcast table
                        nc.vector.tensor_tensor(
                            out=aggT[:, bi * P:(bi + 1) * P], in0=ps,
                            in1=invb_sb[:, b * P:(b + 1) * P],
                            op=mybir.AluOpType.mult,
                        )
                    if "dense" not in parts:
                        continue
                    zp = psZ.tile([D, GCOL], F32, tag="z", name=f"z_{layer}_{g}")
                    nc.tensor.matmul(zp[:, :w], lhsT=wl_sb, rhs=aggT[:, :w],
                                     start=True, stop=False)
                    nc.tensor.matmul(zp[:, :w], lhsT=wr_sb,
                                     rhs=own_sb[:, b0 * P:b0 * P + w],
                                     start=False, stop=True)
                    if layer == 0:
                        nc.scalar.activation(out=hT_sb[:, b0 * P:b0 * P + w],
                                             in_=zp[:, :w], func=func,
                                             bias=bb_sb[:, 0:1], scale=1.0)
                        outT = hT_sb
                    else:
                        outT = gpool.tile([D, GCOL], F32, tag="outT",
                                          name=f"oT_{rep}_{g}")
                        nc.scalar.activation(out=outT[:, :w], in_=zp[:, :w],
                                             func=func, bias=bb_sb[:, 0:1],
                                             scale=1.0)
                    if "store" not in parts:
                        continue
                    for bi in range(nb):
                        b = b0 + bi
                        tp = psT.tile([P, D], F32, tag="tr", name=f"tp_{layer}_{b}")
                        sl = (slice(bi * P, bi * P + P) if layer == 1
                              else slice(b * P, b * P + P))
                        nc.tensor.transpose(out=tp, in_=outT[:, sl],
                                            identity=ident_sb)
                        if layer == 0:
                            nc.scalar.copy(out=nodeh_sb[:, b * D:(b + 1) * D],
                                           in_=tp)
                        else:
                            nc.scalar.copy(out=nodeo_sb[:, b * D:(b + 1) * D],
                                           in_=tp)
                    grp_done[g] = True
                    if layer == 0 and "collective" in parts:
                        # emit any h chunk whose dense groups are all done
                        for c, (cb, csz, g0, g1) in enumerate(cfg.hchunks):
                            if g + 1 == g1 and all(
                                    grp_done.get(gg) for gg in range(g0, g1)):
                                bb0 = cb // P
                                nbk = csz // P
                                hc = hcks[c]
                                nc.sync.dma_start(
                                    out=hc.rearrange(
                                        "(b j) (q f) -> (j q) b f", j=D, q=2),
                                    in_=nodeh_sb.rearrange(
                                        "p (b f) -> p b f",
                                        f=D)[:, bb0:bb0 + nbk, :],
                                )
                                r0 = NC * cb // 2
                                r1 = r0 + NC * csz // 2
                                if one_core:
                                    nc.sync.dma_start(
                                        out=h_full[r0:r0 + csz // 2, :],
                                        in_=hc)
                                else:
                                    nc.gpsimd.collective_compute(
                                        "AllGather",
                                        mybir.AluOpType.bypass,
                                        replica_groups=[list(range(NC))],
                                        ins=[hc[:, :].opt()],
                                        outs=[haggs[c][:, :].opt()],
                                    )
                                    nc.sync.dma_start(
                                        out=h_full[r0:r1, :],
                                        in_=haggs[c])

                if layer == 1 and "store" in parts:
                    nc.sync.dma_start(
                        out=out_dr.ap().rearrange("(b p) f -> p b f", p=P),
                        in_=nodeo_sb.rearrange("p (b f) -> p b f", f=D),
                    )

    nc.compile()
    return nc


def make_in_maps(meta, x, W_l1, b_l1, W_r1, W_l2, b_l2, W_r2):
    cfg = meta.cfg
    x = np.ascontiguousarray(np.asarray(x, dtype=np.float32))
    xp = np.ascontiguousarray(
        x.astype(BF).reshape(cfg.N // 2, 2 * D))
    iota = np.tile(np.repeat(np.arange(P, dtype=np.float32), 2), (P, 1)).astype(BF)
    ident = np.eye(D, dtype=np.float32)
    common = {
        "xp": xp,
        "wl1t": np.ascontiguousarray(np.asarray(W_l1, np.float32).T),
        "wr1t": np.ascontiguousarray(np.asarray(W_r1, np.float32).T),
        "wl2t": np.ascontiguousarray(np.asarray(W_l2, np.float32).T),
        "wr2t": np.ascontiguousarray(np.asarray(W_r2, np.float32).T),
        "b1": np.asarray(b_l1, np.float32).reshape(D, 1).copy(),
        "b2": np.asarray(b_l2, np.float32).reshape(D, 1).copy(),
        "iota": iota,
        "ident": ident,
    }
    in_maps = []
    for k in range(cfg.n_cores):
        xo = x[k * cfg.n_own:(k + 1) * cfg.n_own]
        xoT = np.zeros((D, cfg.n_own_pad), np.float32)
        xoT[:, :cfg.n_own] = xo.T
        in_maps.append(dict(common, xoT=xoT, idx=meta.idx[k],
                            dstf=meta.dstf[k], invb=meta.invb[k]))
    return in_maps


_CACHE = {}
_LAST_RES = None


def kernel(x, edge_index, W_l1, b_l1, W_r1, W_l2, b_l2, W_r2):
    edge_index = np.asarray(edge_index)
    x = np.asarray(x)
    cfg = Cfg(x.shape[0])
    key = hash(edge_index.tobytes())
    if key in _CACHE:
        meta, nc = _CACHE[key]
    else:
        meta = preprocess(edge_index, cfg)
        nc = build_program(meta)
        _CACHE[key] = (meta, nc)
    in_maps = make_in_maps(meta, x, W_l1, b_l1, W_r1, W_l2, b_l2, W_r2)
    res = run_bass_kernel_spmd(nc, in_maps, core_ids=list(range(cfg.n_cores)))
    global _LAST_RES
    _LAST_RES = res
    out = np.concatenate(
        [res.results[k]["out"][:cfg.n_own] for k in range(cfg.n_cores)], axis=0
    )
    return out.astype(np.float32)


# revision 29
# speedup vs baseline: 1.5672x; 1.5672x over previous
"""2-layer GraphSAGE (mean aggregation) on 8 trn2 NeuronCores via Bass/Tile.

Strategy (matches the sharding hint):
  - Nodes are row-sharded across the 8 cores (6250 rows each); edges are
    partitioned by destination core and grouped by 128-node destination
    block.  Messages are fetched with InstDMAGatherAnt from a pair-packed
    bf16 copy of the features: each 256B gather element holds TWO
    consecutive node rows (128B each) and the index is src>>1, so no
    parity split of the edges is needed.  Gather calls round-robin over
    4 SWDGE queues (4 DMA engines).
  - The segment-sum is a one-hot matmul on the tensor engine (bf16).
    Each gathered tile is reduced with TWO matmuls — the even-node
    column half against an even-masked onehot and the odd half against
    an odd-masked onehot — accumulating into one PSUM tile:
        aggT[64f, 128d] += msgs[128e, 64(v)].T @ onehot_v[128e, 128d]
    A single broadcast-AP DVE tensor_tensor(is_equal) against an
    interleaved host-built dstf table builds BOTH onehot variants for a
    whole gather call in one instruction.  Exact f32 1/deg scaling is
    applied at PSUM->SBUF copy time via a host-built broadcast table.
  - The 64x64 weights are replicated; the dense phase runs feature-major
    in f32 on rotating [64, 512] group buffers.
  - h = tanh(layer1) is written pair-packed (128B rows, no padding) and
    AllGathered in node-range chunks that overlap the remaining layer-1
    compute; layer 2 gathers from the chunk-major shared h with the SAME
    edge grouping (pair index pos>>1, parity preserved because all
    layout offsets are even).
"""

import numpy as np
import ml_dtypes

import concourse.bacc as bacc
import concourse.mybir as mybir
import concourse.tile as tile
from concourse.bass import AP
from concourse.bass_utils import run_bass_kernel_spmd

P = 128
D = 64
F32 = mybir.dt.float32
BF16 = mybir.dt.bfloat16
I16 = mybir.dt.int16
BF = ml_dtypes.bfloat16

GCOL = 512  # dense-phase group width (one PSUM bank)


class Cfg:
    def __init__(self, N, n_cores=8, chunk=24, msgs_bufs=6, nqueues=4,
                 oh_bufs=3, n_hchunks=4):
        assert N % n_cores == 0
        self.N = N
        self.n_cores = n_cores
        self.n_own = N // n_cores
        assert self.n_own % 2 == 0
        self.nblk = -(-self.n_own // P)
        self.n_own_pad = self.nblk * P
        self.n_pad_all = self.n_own_pad * n_cores
        self.chunk = chunk
        self.msgs_bufs = msgs_bufs
        self.nqueues = nqueues
        self.oh_bufs = oh_bufs
        # h chunks: contiguous ranges of dense groups (GCOL-node units)
        ngrp = -(-self.nblk * P // GCOL)
        gper = -(-ngrp // n_hchunks)
        self.hchunks = []
        g0 = 0
        while g0 < ngrp:
            g1 = min(g0 + gper, ngrp)
            base = g0 * GCOL
            sz = min(g1 * GCOL, self.n_own_pad) - base
            self.hchunks.append((base, sz, g0, g1))
            g0 = g1
        assert all(b % 2 == 0 and s % 2 == 0 for b, s, _, _ in self.hchunks)
        assert self.n_own_pad // 2 * n_cores <= 32768  # int16 pair idx


class Meta:
    pass


def _wrap16(v):
    """slot i -> [i % 16, i // 16] layout used by dma_gather idx tables."""
    assert v.shape[0] % 16 == 0
    return np.ascontiguousarray(v.reshape(-1, 16).T)


def preprocess(edge_index, cfg):
    """Partition/group edges; build per-core gather index + onehot tables."""
    src = np.asarray(edge_index[0], dtype=np.int64)
    dst = np.asarray(edge_index[1], dtype=np.int64)
    E = src.shape[0]
    NC, NBLK = cfg.n_cores, cfg.nblk

    cnt = np.bincount(dst, minlength=cfg.N).astype(np.float32)
    inv = (1.0 / np.maximum(cnt, 1.0)).astype(np.float32)

    core = dst // cfg.n_own
    dstl = dst - core * cfg.n_own
    blk = dstl // P
    inb = dstl - blk * P
    par = (src & 1).astype(np.int64)

    key = core * NBLK + blk
    gcnt = np.bincount(key, minlength=NC * NBLK).reshape(NC, NBLK)
    # uniform (max over cores) tile counts per block
    TB = np.maximum(1, -(-gcnt.max(axis=0) // P))
    toff = np.concatenate([[0], np.cumsum(TB)])
    T_ALL = int(toff[-1])

    # rank of each edge within its (core, blk) group
    order = np.argsort(key, kind="stable")
    gstart = np.concatenate(
        [[0], np.cumsum(np.bincount(key, minlength=NC * NBLK))])[:-1]
    rank = np.empty(E, dtype=np.int64)
    rank[order] = np.arange(E) - gstart[key[order]]
    slot = toff[blk] * P + rank
    tile_of = slot >> 7

    # h-space (chunk-major, padded) position of each source node
    src_core = src // cfg.n_own
    local = src - src_core * cfg.n_own
    bases = np.array([b for b, _, _, _ in cfg.hchunks], dtype=np.int64)
    sizes = np.array([s for _, s, _, _ in cfg.hchunks], dtype=np.int64)
    ci = np.searchsorted(bases, local, side="right") - 1
    pos = NC * bases[ci] + src_core * sizes[ci] + (local - bases[ci])

    # SPMD-uniform per-(tile, parity-variant) activity across all cores
    act = np.zeros((T_ALL, 2), dtype=bool)
    act[tile_of, par] = True

    meta = Meta()
    meta.cfg = cfg
    meta.T_ALL = T_ALL
    meta.toff = toff
    meta.block_tiles = []
    for b in range(NBLK):
        bt = [(int(t), v) for t in range(int(toff[b]), int(toff[b + 1]))
              for v in (0, 1) if act[t, v]]
        if not bt:
            bt = [(int(toff[b]), 0)]
        meta.block_tiles.append(bt)

    # per-core tables
    meta.idx = []    # [128, 8*T_ALL*2] int16 : layer1 | layer2 (pair indices)
    meta.dstf = []   # [128, 2*T_ALL] bf16, col 2t+v = onehot targets, -1 pad
    meta.invb = []   # [64, n_own_pad] f32 : 1/deg broadcast down 64 partitions
    for k in range(NC):
        m = core == k
        sl = slot[m]
        i1 = np.zeros(T_ALL * P, np.int16)
        i2 = np.zeros(T_ALL * P, np.int16)
        i1[sl] = src[m] >> 1
        i2[sl] = pos[m] >> 1
        w = np.concatenate([_wrap16(i1), _wrap16(i2)], axis=1)
        # the gather ucode reads each Q7 core's idx stripe from its own
        # 16-partition group -> replicate 8x down the partition axis
        meta.idx.append(np.ascontiguousarray(np.tile(w, (8, 1))))

        df = np.full(2 * T_ALL * P, -1.0, BF)
        gi = (sl >> 7) * 2 + par[m]
        df[gi * P + (sl & (P - 1))] = inb[m].astype(BF)
        meta.dstf.append(np.ascontiguousarray(df.reshape(2 * T_ALL, P).T))

        iv = np.ones(cfg.n_own_pad, np.float32)
        iv[:cfg.n_own] = inv[k * cfg.n_own:(k + 1) * cfg.n_own]
        meta.invb.append(np.ascontiguousarray(np.tile(iv, (D, 1))))

    # dma_gather calls: chunks of tiles, annotated with the first block
    calls = []
    t0 = 0
    while t0 < T_ALL:
        nt = min(cfg.chunk, T_ALL - t0)
        fb = int(np.searchsorted(toff, t0, side="right") - 1)
        calls.append((t0, nt, fb))
        t0 += nt
    meta.calls = calls
    return meta


def build_program(meta, one_core=False,
                  parts=("gather", "agg", "dense", "store", "collective"),
                  reps=1):
    cfg = meta.cfg
    NC, NBLK = cfg.n_cores, cfg.nblk
    NP = cfg.n_own_pad
    BPG = GCOL // P  # blocks per dense group
    nc = bacc.Bacc(
        "TRN2", target_bir_lowering=False, debug=False,
        num_devices=1 if one_core else NC,
        num_swdge_queues=cfg.nqueues,
    )

    xp_dr = nc.dram_tensor("xp", [cfg.N // 2, P], BF16, kind="ExternalInput")
    xoT_dr = nc.dram_tensor("xoT", [D, NP], F32, kind="ExternalInput")
    idx_dr = nc.dram_tensor("idx", list(meta.idx[0].shape), I16, kind="ExternalInput")
    dstf_dr = nc.dram_tensor("dstf", [P, 2 * meta.T_ALL], BF16, kind="ExternalInput")
    invb_dr = nc.dram_tensor("invb", [D, NP], F32, kind="ExternalInput")
    wl1_dr = nc.dram_tensor("wl1t", [D, D], F32, kind="ExternalInput")
    wr1_dr = nc.dram_tensor("wr1t", [D, D], F32, kind="ExternalInput")
    wl2_dr = nc.dram_tensor("wl2t", [D, D], F32, kind="ExternalInput")
    wr2_dr = nc.dram_tensor("wr2t", [D, D], F32, kind="ExternalInput")
    b1_dr = nc.dram_tensor("b1", [D, 1], F32, kind="ExternalInput")
    b2_dr = nc.dram_tensor("b2", [D, 1], F32, kind="ExternalInput")
    iota_dr = nc.dram_tensor("iota", [P, 2 * P], BF16, kind="ExternalInput")
    id_dr = nc.dram_tensor("ident", [D, D], F32, kind="ExternalInput")
    out_dr = nc.dram_tensor("out", [NP, D], F32, kind="ExternalOutput")

    with tile.TileContext(nc) as tc:
        with (
            tc.tile_pool(name="const", bufs=1) as cpool,
            tc.tile_pool(name="big", bufs=1) as bpool,
            tc.tile_pool(name="msgs", bufs=cfg.msgs_bufs) as mpool,
            tc.tile_pool(name="idxp", bufs=8) as ipool,
            tc.tile_pool(name="ohp", bufs=cfg.oh_bufs) as ohpool,
            tc.tile_pool(name="grp", bufs=2) as gpool,
            tc.tile_pool(name="psA", bufs=4, space="PSUM") as psA,
            tc.tile_pool(name="psZ", bufs=2, space="PSUM") as psZ,
            tc.tile_pool(name="psT", bufs=2, space="PSUM") as psT,
            tc.tile_pool(name="dram", bufs=1, space="DRAM") as dpool,
        ):
            def load(pool, dr, shape, name, dt=F32, tag=""):
                t = pool.tile(shape, dt, name=name, tag=tag or name)
                nc.sync.dma_start(out=t, in_=dr.ap())
                return t

            iota_sb = load(cpool, iota_dr, [P, 2 * P], "iota_sb", dt=BF16)
            ident_sb = load(cpool, id_dr, [D, D], "ident_sb")
            wl1_sb = load(cpool, wl1_dr, [D, D], "wl1_sb")
            wr1_sb = load(cpool, wr1_dr, [D, D], "wr1_sb")
            wl2_sb = load(cpool, wl2_dr, [D, D], "wl2_sb")
            wr2_sb = load(cpool, wr2_dr, [D, D], "wr2_sb")
            b1_sb = load(cpool, b1_dr, [D, 1], "b1_sb")
            b2_sb = load(cpool, b2_dr, [D, 1], "b2_sb")
            dstf_sb = load(bpool, dstf_dr, [P, 2 * meta.T_ALL], "dstf_sb", dt=BF16)
            invb_sb = load(bpool, invb_dr, [D, NP], "invb_sb")
            xoT_sb = load(bpool, xoT_dr, [D, NP], "xoT_sb")
            hT_sb = bpool.tile([D, NP], F32, name="hT_sb")
            nodeh_sb = bpool.tile([P, NBLK * D], BF16, name="nodeh_sb")
            nodeo_sb = bpool.tile([P, NBLK * D], F32, name="nodeo_sb")

            for rep in range(reps):
              h_full = dpool.tile([cfg.n_pad_all // 2, P], BF16,
                                  name=f"h_full_{rep}", tag=f"hf{rep}")
              hcks = []
              haggs = []
              for c, (base, sz, _, _) in enumerate(cfg.hchunks):
                  hcks.append(dpool.tile([sz // 2, P], BF16,
                                         name=f"h_c_{rep}_{c}", tag=f"hc{rep}_{c}"))
                  haggs.append(dpool.tile([NC * sz // 2, P], BF16,
                                          name=f"h_a_{rep}_{c}",
                                          tag=f"ha{rep}_{c}",
                                          addr_space="Shared"))
              for layer in range(2):
                if layer == 0:
                    gsrc = xp_dr.ap()
                    off = 0
                else:
                    gsrc = h_full[:, :]
                    off = meta.T_ALL * 8

                # ---- gather messages (256B elems = node pairs) ----
                tsrc = {}
                ohsrc = {}
                for ci, (t0, ntile, _fb) in enumerate(meta.calls):
                    mt = mpool.tile([P, cfg.chunk, P], BF16, tag="msgs",
                                    name=f"m_{layer}_{ci}")
                    if "gather" in parts:
                        it = ipool.tile([P, cfg.chunk * 8], I16, tag="idx",
                                        name=f"i_{layer}_{ci}")
                        cols = ntile * 8
                        coff = off + t0 * 8
                        nc.sync.dma_start(out=it[:, :cols],
                                          in_=idx_dr.ap()[:, coff:coff + cols])
                        nc.gpsimd.dma_gather(
                            mt[:, :ntile, :],
                            gsrc,
                            it[:, :cols],
                            num_idxs=ntile * P,
                            num_idxs_reg=ntile * P,
                            elem_size=P,
                            single_packet=False,
                            queue_num=ci % cfg.nqueues,
                        )
                    # one broadcast-AP DVE op builds BOTH onehot variants for
                    # this whole chunk, variant-interleaved so every operand's
                    # innermost dim is packed (keeps the DVE 2x 16-bit mode):
                    #   oh[e, t, d, v] = (iota2[e, 2d+v] == dstf[e, 2(t0+t)+v])
                    oht = ohpool.tile([P, cfg.chunk, P, 2], BF16, tag="oh",
                                      name=f"oh_{layer}_{ci}")
                    if "agg" in parts:
                        io = iota_sb[:, :]
                        in0 = AP(io.tensor, io.offset,
                                 [io.ap[0], [0, ntile], [2, P], [1, 2]])
                        df = dstf_sb[:, 2 * t0:2 * (t0 + ntile)]
                        in1 = AP(df.tensor, df.offset,
                                 [df.ap[0], [2, ntile], [0, P], [1, 2]])
                        nc.vector.tensor_tensor(
                            out=oht[:, :ntile, :, :], in0=in0, in1=in1,
                            op=mybir.AluOpType.is_equal,
                        )
                    for j in range(ntile):
                        tsrc[t0 + j] = (mt, j)
                        ohsrc[t0 + j] = (oht, j)

                # ---- blocks: onehot matmul segment-sum + dense per group ----
                if layer == 0:
                    wl_sb, wr_sb, bb_sb = wl1_sb, wr1_sb, b1_sb
                    own_sb = xoT_sb
                    func = mybir.ActivationFunctionType.Tanh
                else:
                    wl_sb, wr_sb, bb_sb = wl2_sb, wr2_sb, b2_sb
                    own_sb = hT_sb
                    func = mybir.ActivationFunctionType.Identity

                ngrp = -(-NBLK // BPG)
                grp_done = {}
                for g in range(ngrp if "agg" in parts else 0):
                    b0 = g * BPG
                    nb = min(BPG, NBLK - b0)
                    w = nb * P
                    aggT = gpool.tile([D, GCOL], F32, tag="aggT",
                                      name=f"agg_{rep}_{layer}_{g}")
                    for bi in range(nb):
                        b = b0 + bi
                        ps = psA.tile([D, P], F32, tag="agg", name=f"ps_{layer}_{b}")
                        gts = meta.block_tiles[b]
                        for j, (gt, v) in enumerate(gts):
                            mt, lt = tsrc[gt]
                            oht, lo = ohsrc[gt]
                            nc.tensor.matmul(
                                ps, lhsT=mt[:, lt, v * D:v * D + D],
                                rhs=oht[:, lo, :, v],
                                start=(j == 0), stop=(j == len(gts) - 1),
                            )
                        # exact mean scaling: psum * (1/deg) broadcast table
                        nc.vector.tensor_tensor(
                            out=aggT[:, bi * P:(bi + 1) * P], in0=ps,
                            in1=invb_sb[:, b * P:(b + 1) * P],
                            op=mybir.AluOpType.mult,
                        )
                    if "dense" not in parts:
                        continue
                    zp = psZ.tile([D, GCOL], F32, tag="z", name=f"z_{layer}_{g}")
                    nc.tensor.matmul(zp[:, :w], lhsT=wl_sb, rhs=aggT[:, :w],
                                     start=True, stop=False)
                    nc.tensor.matmul(zp[:, :w], lhsT=wr_sb,
                                     rhs=own_sb[:, b0 * P:b0 * P + w],
                                     start=False, stop=True)
                    if layer == 0:
                        nc.scalar.activation(out=hT_sb[:, b0 * P:b0 * P + w],
                                             in_=zp[:, :w], func=func,
                                             bias=bb_sb[:, 0:1], scale=1.0)
                        outT = hT_sb
                    else:
                        outT = gpool.tile([D, GCOL], F32, tag="outT",
                                          name=f"oT_{rep}_{g}")
                        nc.scalar.activation(out=outT[:, :w], in_=zp[:, :w],
                                             func=func, bias=bb_sb[:, 0:1],
                                             scale=1.0)
                    if "store" not in parts:
                        continue
                    for bi in range(nb):
                        b = b0 + bi
                        tp = psT.tile([P, D], F32, tag="tr", name=f"tp_{layer}_{b}")
                        sl = (slice(bi * P, bi * P + P) if layer == 1
                              else slice(b * P, b * P + P))
                        nc.tensor.transpose(out=tp, in_=outT[:, sl],
                                            identity=ident_sb)
                        if layer == 0:
                            nc.scalar.copy(out=nodeh_sb[:, b * D:(b + 1) * D],
                                           in_=tp)
                        else:
                            nc.scalar.copy(out=nodeo_sb[:, b * D:(b + 1) * D],
                                           in_=tp)
                    grp_done[g] = True
                    if layer == 0 and "collective" in parts:
                        # emit any h chunk whose dense groups are all done
                        for c, (cb, csz, g0, g1) in enumerate(cfg.hchunks):
                            if g + 1 == g1 and all(
                                    grp_done.get(gg) for gg in range(g0, g1)):
                                bb0 = cb // P
                                nbk = csz // P
                                hc = hcks[c]
                                nc.sync.dma_start(
                                    out=hc.rearrange(
                                        "(b j) (q f) -> (j q) b f", j=D, q=2),
                                    in_=nodeh_sb.rearrange(
                                        "p (b f) -> p b f",
                                        f=D)[:, bb0:bb0 + nbk, :],
                                )
                                r0 = NC * cb // 2
                                r1 = r0 + NC * csz // 2
                                if one_core:
                                    nc.sync.dma_start(
                                        out=h_full[r0:r0 + csz // 2, :],
                                        in_=hc)
                                else:
                                    nc.gpsimd.collective_compute(
                                        "AllGather",
                                        mybir.AluOpType.bypass,
                                        replica_groups=[list(range(NC))],
                                        ins=[hc[:, :].opt()],
                                        outs=[haggs[c][:, :].opt()],
                                    )
                                    nc.sync.dma_start(
                                        out=h_full[r0:r1, :],
                                        in_=haggs[c])

                if layer == 1 and "store" in parts:
                    nc.sync.dma_start(
                        out=out_dr.ap().rearrange("(b p) f -> p b f", p=P),
                        in_=nodeo_sb.rearrange("p (b f) -> p b f", f=D),
                    )

    nc.compile()
    return nc


def make_in_maps(meta, x, W_l1, b_l1, W_r1, W_l2, b_l2, W_r2):
    cfg = meta.cfg
    x = np.ascontiguousarray(np.asarray(x, dtype=np.float32))
    xp = np.ascontiguousarray(
        x.astype(BF).reshape(cfg.N // 2, 2 * D))
    iota = np.tile(np.repeat(np.arange(P, dtype=np.float32), 2), (P, 1)).astype(BF)
    ident = np.eye(D, dtype=np.float32)
    common = {
        "xp": xp,
        "wl1t": np.ascontiguousarray(np.asarray(W_l1, np.float32).T),
        "wr1t": np.ascontiguousarray(np.asarray(W_r1, np.float32).T),
        "wl2t": np.ascontiguousarray(np.asarray(W_l2, np.float32).T),
        "wr2t": np.ascontiguousarray(np.asarray(W_r2, np.float32).T),
        "b1": np.asarray(b_l1, np.float32).reshape(D, 1).copy(),
        "b2": np.asarray(b_l2, np.float32).reshape(D, 1).copy(),
        "iota": iota,
        "ident": ident,
    }
    in_maps = []
    for k in range(cfg.n_cores):
        xo = x[k * cfg.n_own:(k + 1) * cfg.n_own]
        xoT = np.zeros((D, cfg.n_own_pad), np.float32)
        xoT[:, :cfg.n_own] = xo.T
        in_maps.append(dict(common, xoT=xoT, idx=meta.idx[k],
                            dstf=meta.dstf[k], invb=meta.invb[k]))
    return in_maps


_CACHE = {}
_LAST_RES = None


def kernel(x, edge_index, W_l1, b_l1, W_r1, W_l2, b_l2, W_r2):
    edge_index = np.asarray(edge_index)
    x = np.asarray(x)
    cfg = Cfg(x.shape[0])
    key = hash(edge_index.tobytes())
    if key in _CACHE:
        meta, nc = _CACHE[key]
    else:
        meta = preprocess(edge_index, cfg)
        nc = build_program(meta)
        _CACHE[key] = (meta, nc)
    in_maps = make_in_maps(meta, x, W_l1, b_l1, W_r1, W_l2, b_l2, W_r2)
    res = run_bass_kernel_spmd(nc, in_maps, core_ids=list(range(cfg.n_cores)))
    global _LAST_RES
    _LAST_RES = res
    out = np.concatenate(
        [res.results[k]["out"][:cfg.n_own] for k in range(cfg.n_cores)], axis=0
    )
    return out.astype(np.float32)
